# revision 16
# baseline (speedup 1.0000x reference)
"""Single-head causal attention on 8 TRN2 NeuronCores (Bass/Tile), v3.

Problem: x [4, 2048, 1024] fp32; wq/wk/wv [1024, 128]; wo [128, 1024].
out = softmax_causal((x@wq)(x@wk)^T / sqrt(128)) @ (x@wv) @ wo

Sharding: 8 cores = 4 batches x 2 query-interleavings (causal-load-balanced
"zebra": per group of 4 seq blocks, even core takes {4g, 4g+3}, odd core
{4g+1, 4g+2}). Each core's x arrives transposed and column-permuted; slot j
(256 queries) attends to permuted key prefix [0 : 512*(j+1)] with a static
multiplicative fp16 mask on the diagonal 512-key group.

Key structure (from NTFF profiling of v1/v2):
- Inputs stream on all 3 HWDGE/SWDGE queues (sync/scalar/gpsimd), one
  512KB chunk per transfer, interleaved so each chunk lands just before
  the d-chunk-outer projection loop consumes it (~190 GB/s per queue).
- ~3us of dummy N=64 matmuls during the DMA wait trip the HAM clock gate
  so real matmuls run at 2.4 GHz from the first projection.
- Attention streams 512 cols/matmul over slot pairs (3,2) then (1,0) --
  LDWEIGHTS (~104ns) hides under the 213ns rhs stream. Solo (diagonal)
  groups store two 256-wide blocks contiguously so ACT/DVE ops stay 2D.
- V[k,h] comes from two dma_start_transpose calls (xbar), zero PE cost.
- den: ones-lhsT matmuls for shared groups (evacuated + transposed via
  4 small DMAs + reciprocal EARLY, off the critical path); the final solo
  group of each pair instead uses transposed matmuls (lhsT=PT block,
  rhs=ones -> [128q, 1]) so no [1,N] transpose sits on the tail.
- Output fp16; out-projection PSUM reuses the VT accumulator banks.

PSUM budget: P1 = warm 1 + qt 2 + kt 4 = 7 banks.
P2 = vt/out 2 + st 2x2 + den 1 + ctx 1 = 8 banks.
"""

import numpy as np

import concourse.bass as bass
from concourse import bacc
import concourse.mybir as mybir
import concourse.tile as tile
from concourse.bass_utils import run_bass_kernel_spmd
from concourse.masks import make_identity

F32 = mybir.dt.float32
F16 = mybir.dt.float16
F8 = mybir.dt.float8e4

D_MODEL = 1024
D_HEAD = 128
SEQ = 2048
BATCH = 4
NCORES = 8
P = 128            # partitions / block size
DC = D_MODEL // P  # 8 d_model chunks
NB = SEQ // P      # 16 seq blocks
NSLOT = 4          # query slots per core
QW = 256           # queries per slot
NQ = NSLOT * QW    # 1024 queries per core
SCALE = 1.0 / float(np.sqrt(D_HEAD))
EXP_BIAS = -6.5    # exp(scale*s - 6.5) <= e^5 ~ 148 < 240 (fp8e4-safe; max scaled score ~11.3)

# womask column offsets (fp16, [128, 4096])
OFF_WO = 0
OFF_MP = 1024            # pair mask   [128, 4*512]
OFF_MS = 3072            # solo mask   [128, 4*256]


def block_order(parity: int) -> list[int]:
    order = []
    for g in range(4):
        if parity == 0:
            order += [4 * g, 4 * g + 3, 4 * g + 1, 4 * g + 2]
        else:
            order += [4 * g + 1, 4 * g + 2, 4 * g, 4 * g + 3]
    return order


def make_mask01(parity: int) -> np.ndarray:
    """Multiplicative 0/1 mask for the diagonal 512-key group, applied to
    PT (post-exp), transposed: [512 k, 256 q] fp16."""
    P4 = block_order(parity)[:4]
    m = np.zeros((512, 256), dtype=np.float16)
    kr = np.arange(P)[:, None]
    qc = np.arange(P)[None, :]
    tri = (kr <= qc).astype(np.float16)
    for kb2 in range(4):
        K = P4[kb2]
        for qb2 in range(2):
            Q = P4[qb2]
            blk = m[P * kb2:P * (kb2 + 1), P * qb2:P * (qb2 + 1)]
            if K < Q:
                blk[:] = 1.0
            elif K > Q:
                blk[:] = 0.0
            else:
                blk[:] = tri
    return m


def _attention_kernel(tc: tile.TileContext, xt_d, wqk_d, wv_d, womask_d,
                      out_d):
    nc = tc.nc

    with (
        tc.tile_pool(name="const", bufs=1) as const_pool,
        tc.tile_pool(name="big", bufs=1) as big_pool,
        tc.tile_pool(name="outp", bufs=3) as out_pool,
    ):
        # ---- staggered input DMAs on 3 queues; chunk k lands just before
        # the projection loop needs it ----
        wqk_sb = const_pool.tile([P, 2048], F16)
        wv_sb = const_pool.tile([P, 1024], F16)
        womask_sb = const_pool.tile([P, 4096], F16)
        xt_sb = big_pool.tile([P, DC, SEQ], F16)

        def xchunk(queue, c, h):
            queue.dma_start(
                out=xt_sb[:, c, 1024 * h:1024 * (h + 1)],
                in_=xt_d[P * c:P * (c + 1), 1024 * h:1024 * (h + 1)])

        # Queue assignment follows measured per-queue rates (~120-140 GB/s
        # each, concurrent; SWDGE delivers from ~16us). 0.25MB granularity
        # keeps every PE input-stall under the ~3.4us HAM re-throttle
        # window. Consumption order is (c,h) lexicographic.
        nc.sync.dma_start(out=wqk_sb, in_=wqk_d)
        for c, h, q in [(0, 0, nc.scalar), (0, 1, nc.sync),
                        (1, 0, nc.scalar), (1, 1, nc.sync),
                        (2, 0, nc.scalar), (2, 1, nc.sync),
                        (3, 0, nc.gpsimd), (3, 1, nc.scalar),
                        (4, 0, nc.sync), (4, 1, nc.gpsimd),
                        (5, 0, nc.scalar), (5, 1, nc.sync),
                        (6, 0, nc.gpsimd), (6, 1, nc.scalar),
                        (7, 0, nc.sync), (7, 1, nc.scalar)]:
            xchunk(q, c, h)
            if (c, h) == (2, 1):
                nc.sync.dma_start(out=wv_sb, in_=wv_d)
        nc.scalar.dma_start(out=womask_sb, in_=womask_d)
        # dummy exp pinned behind the wqk DMA (dep forces the scheduler to
        # keep it -- and the ~1.5us ACT table load walrus attaches to it --
        # after the scalar-queue input issues instead of hoisting both to
        # the queue head, which would delay c0)
        expbias = const_pool.tile([P, 1], F32)
        nc.vector.memset(expbias, EXP_BIAS)
        actwarm = const_pool.tile([P, 1], F32)
        nc.scalar.activation(out=actwarm, in_=wqk_sb[:, 0:1],
                             func=mybir.ActivationFunctionType.Exp)

        wq_c = wqk_sb[:, 0:1024].rearrange("p (c h) -> p c h", h=P)
        wk_c = wqk_sb[:, 1024:2048].rearrange("p (c h) -> p c h", h=P)
        wv_c = wv_sb.rearrange("p (c h) -> p c h", h=P)
        wo_sb = womask_sb[:, OFF_WO:OFF_WO + 1024]
        maskp3 = womask_sb[:, OFF_MP:OFF_MP + 2048].rearrange(
            "p (b w) -> p b w", w=512)   # additive 0/-30000, [128, 4, 512]
        masks3 = womask_sb[:, OFF_MS:OFF_MS + 1024].rearrange(
            "p (b w) -> p b w", w=QW)    # additive, [128, 4, 256]

        # ---- constants ----
        ones = const_pool.tile([P, 1], F16)
        nc.vector.memset(ones, 1.0)
        ident = const_pool.tile([P, P], F16)
        make_identity(nc, ident)
        warm_w = const_pool.tile([P, 64], F16)
        nc.vector.memset(warm_w, 0.0)

        qt_sb = big_pool.tile([P, NQ], F16)
        kt_sb = big_pool.tile([P, SEQ], F16)
        vt_sb = big_pool.tile([P, SEQ], F16)
        v_sb = big_pool.tile([P, NB, P], F16)   # V[k,h]; block kb at [:, kb, :]
        ctxt_sb = big_pool.tile([P, NQ], F16)
        # flat PT slabs; shared group g: block kb at [512kb : 512kb+512];
        # solo groups: two 256-wide blocks packed per 512 entry
        pt32_sb = big_pool.tile([P, 16 * 512], F16)
        pt10_sb = big_pool.tile([P, 8 * 512], F16)
        den32_sb = big_pool.tile([1, 512], F32)
        den10_sb = big_pool.tile([1, 512], F32)
        dent = big_pool.tile([P, 8], F32)     # den, col = out block qbi
        dentmm = big_pool.tile([P, 8], F32)   # solo-group den part
        rden = big_pool.tile([P, 8], F32)

        def pt_off(pair_ngs, kb):
            """Column offset of block kb in its pair's PT slab."""
            nsh = 4 * pair_ngs   # blocks in shared groups
            if kb < nsh:
                return 512 * kb
            return 512 * nsh + QW * (kb - nsh)

        # ---- phase 1: PE warm-up burst, then QT + KT (d-chunk outer) ----
        with tc.tile_pool(name="p1", bufs=1, space="PSUM") as p1:
            warm_ps = p1.tile([P, 64], F32, tag="warm")
            for _ in range(100):
                nc.tensor.matmul(warm_ps[0:64, :], lhsT=warm_w, rhs=warm_w,
                                 start=True, stop=True, skip_group_check=True)
            qt_ps = [p1.tile([P, 512], F32, name=f"qt_ps{i}", tag=f"qt{i}")
                     for i in range(2)]
            kt_ps = [p1.tile([P, 512], F32, name=f"kt_ps{i}", tag=f"kt{i}")
                     for i in range(4)]
            for c in range(DC):
                xc = xt_sb[:, c, :]
                xr = xc.rearrange("p (g q) -> p g q", q=QW)
                # per half-chunk: one QT + two KT matmuls (matches the
                # 0.25MB DMA granularity)
                for h in range(2):
                    nc.tensor.matmul(qt_ps[h], lhsT=wq_c[:, c, :],
                                     rhs=xr[:, 4 * h:4 * h + 3:2, :],
                                     start=(c == 0), stop=(c == DC - 1),
                                     skip_group_check=True)
                    for t in (2 * h, 2 * h + 1):
                        nc.tensor.matmul(kt_ps[t], lhsT=wk_c[:, c, :],
                                         rhs=xc[:, 512 * t:512 * (t + 1)],
                                         start=(c == 0), stop=(c == DC - 1),
                                         skip_group_check=True)
            # split evacuations across DVE and ACT: the next PSUM pool's
            # first tiles gate on ALL of these (pool boundary)
            nc.vector.tensor_copy(qt_sb[:, 0:512], qt_ps[0])
            nc.scalar.copy(qt_sb[:, 512:1024], qt_ps[1])
            for t in range(4):
                if t % 2 == 0:
                    nc.vector.tensor_copy(kt_sb[:, 512 * t:512 * (t + 1)],
                                          kt_ps[t])
                else:
                    nc.scalar.copy(kt_sb[:, 512 * t:512 * (t + 1)], kt_ps[t])

        # ---- phase 2: VT (tile-outer) + xbar V-transpose + paired attention
        with tc.tile_pool(name="p2", bufs=1, space="PSUM") as p2:

            def vt_tile(t):
                ps = p2.tile([P, 512], F32, tag="vtop", bufs=2,
                             name=f"vt_ps{t}")
                for c in range(DC):
                    nc.tensor.matmul(ps, lhsT=wv_c[:, c, :],
                                     rhs=xt_sb[:, c, 512 * t:512 * (t + 1)],
                                     start=(c == 0), stop=(c == DC - 1),
                                     skip_group_check=True)
                nc.vector.tensor_copy(vt_sb[:, 512 * t:512 * (t + 1)], ps)
                if t % 2 == 1:
                    # V[k,h] for blocks 8t' .. 8t'+7 in one xbar call
                    tt = t // 2
                    nc.sync.dma_start_transpose(
                        out=v_sb[:, 8 * tt:8 * tt + 8, :],
                        in_=vt_sb[:, 1024 * tt:1024 * (tt + 1)])

            # pair = (qlo, ngs, pt, qbase)
            pair32 = (512, 3, pt32_sb, 4)
            pair10 = (0, 1, pt10_sb, 0)

            def st_exp(pair, g):
                """Scores (+ additive causal mask on the diagonal group,
                via identity-weighted matmuls) + exp, fp8 out."""
                qlo, ngs, pt, _ = pair
                solo = g >= ngs
                diag_p = g == ngs - 1   # shared diagonal: lower slot masked
                diag_s = solo           # solo diagonal: upper slot masked
                rhs = qt_sb[:, qlo + QW:qlo + 512] if solo \
                    else qt_sb[:, qlo:qlo + 512]
                for half in range(2):
                    st = p2.tile([P, 1024], F32, tag="st", bufs=2,
                                 name=f"st{pair[0]}_{g}_{half}")
                    for k2 in range(2):
                        kb = 4 * g + 2 * half + k2
                        w = QW if solo else 512
                        # solo: both MMs share one bank -- only the first
                        # may carry start=True (bank-wide has_written clear)
                        nc.tensor.matmul(st[:, w * k2:w * (k2 + 1)],
                                         lhsT=kt_sb[:, P * kb:P * (kb + 1)],
                                         rhs=rhs,
                                         start=(k2 == 0) if solo else True,
                                         stop=not (diag_p or diag_s),
                                         skip_group_check=True)
                    if diag_p:
                        for k2 in range(2):
                            nc.tensor.matmul(
                                st[:, 512 * k2:512 * (k2 + 1)],
                                lhsT=ident,
                                rhs=maskp3[:, 2 * half + k2, :],
                                start=False, stop=(k2 == 1),
                                skip_group_check=True)
                    if diag_s:
                        nc.tensor.matmul(
                            st[:, 0:512], lhsT=ident,
                            rhs=masks3[:, 2 * half:2 * half + 2, :],
                            start=False, stop=True,
                            skip_group_check=True)
                    kb0 = 4 * g + 2 * half
                    ww = 512 if solo else 1024   # contiguous in both cases
                    nc.scalar.activation(
                        out=pt[:, pt_off(ngs, kb0):pt_off(ngs, kb0) + ww],
                        in_=st[:, 0:ww],
                        func=mybir.ActivationFunctionType.Exp,
                        bias=expbias, scale=SCALE)

            def den_grp(pair, den_t, g, nkb):
                """ones-lhsT den matmuls (solo groups hit only the upper
                slot's 256 columns)."""
                _, ngs, pt, _ = pair
                solo = g >= ngs
                w = QW if solo else 512
                co = QW if solo else 0
                for k2 in range(4):
                    kb = 4 * g + k2
                    o = pt_off(ngs, kb)
                    nc.tensor.matmul(den_t[0:1, co:co + w], lhsT=ones,
                                     rhs=pt[:, o:o + w],
                                     start=(kb == 0), stop=(kb == nkb - 1),
                                     skip_group_check=True)

            def ctx_grp(pair, ctx_t, g):
                _, ngs, pt, _ = pair
                nkb = 4 * (ngs + 1)
                solo = g >= ngs
                w = QW if solo else 512
                co = QW if solo else 0
                for k2 in range(4):
                    kb = 4 * g + k2
                    o = pt_off(ngs, kb)
                    nc.tensor.matmul(ctx_t[:, co:co + w],
                                     lhsT=v_sb[:, kb, :],
                                     rhs=pt[:, o:o + w],
                                     start=(kb == 0), stop=(kb == nkb - 1),
                                     skip_group_check=True)

            def den_shared_finish(pair, den_t, den_row_sb):
                """Evacuate shared-group den sums, transpose via 4 small
                DMAs, reciprocal for the lower slot (fully summed)."""
                qbase = pair[3]
                nc.vector.tensor_copy(den_row_sb, den_t[0:1, 0:512])
                for m in range(4):
                    nc.sync.dma_start(
                        out=dent[:, qbase + m:qbase + m + 1],
                        in_=den_row_sb[:, P * m:P * (m + 1)])
                nc.vector.reciprocal(rden[:, qbase:qbase + 2],
                                     dent[:, qbase:qbase + 2])

            def den_solo(pair, dent_t):
                """Transposed den matmuls for the pair's solo group:
                lhsT = PT [128k, 128q], rhs = ones -> [128q, 1]."""
                _, ngs, pt, _ = pair
                g = ngs   # solo group index: pair32 g=3, pair10 g=1
                for qc in range(2):
                    for k2 in range(4):
                        kb = 4 * g + k2
                        o = pt_off(ngs, kb) + P * qc
                        nc.tensor.matmul(dent_t[:, qc:qc + 1],
                                         lhsT=pt[:, o:o + P],
                                         rhs=ones,
                                         start=(k2 == 0), stop=(k2 == 3),
                                         skip_group_check=True)

            def den_solo_finish(pair, dent_t):
                hi = pair[3] + 2
                nc.vector.tensor_copy(dentmm[:, hi:hi + 2], dent_t[:, 0:2])
                nc.vector.tensor_add(dent[:, hi:hi + 2], dent[:, hi:hi + 2],
                                     dentmm[:, hi:hi + 2])
                nc.vector.reciprocal(rden[:, hi:hi + 2], dent[:, hi:hi + 2])

            def ctx_finish(pair, ctx_t):
                qlo = pair[0]
                nc.scalar.copy(ctxt_sb[:, qlo:qlo + 512], ctx_t)

            out_seq = [0]

            def out_block(qbi, dma_queue, tag=None):
                """Output projection for 128-query block qbi, scaled by
                1/den (whole-block evac, DVE/ACT alternating), fp16, DMA."""
                ot = out_pool.tile([P, D_MODEL], F16, tag="ot")
                rd = rden[:, qbi:qbi + 1]
                ps = p2.tile([P, 1024], F32, tag="st", bufs=2,
                             name=f"op{qbi}")
                for h in range(2):
                    nc.tensor.matmul(ps[:, 512 * h:512 * (h + 1)],
                                     lhsT=ctxt_sb[:, P * qbi:P * (qbi + 1)],
                                     rhs=wo_sb[:, 512 * h:512 * (h + 1)],
                                     start=True, stop=True,
                                     skip_group_check=True)
                if out_seq[0] % 2 == 0:
                    nc.vector.tensor_scalar_mul(ot, ps, rd)
                else:
                    nc.scalar.mul(ot, ps, rd)
                out_seq[0] += 1
                dma_queue.dma_start(out=out_d[P * qbi:P * (qbi + 1), :], in_=ot)

            # ---- interleaved emission (PE order) ----
            vt_tile(0)
            st_exp(pair32, 0)
            vt_tile(1)
            st_exp(pair32, 1)
            den32_t = p2.tile([P, 512], F32, tag="den", name="den32")
            ctx32_t = p2.tile([P, 512], F32, tag="ctx", name="ctx32")
            den_grp(pair32, den32_t, 0, 16)
            ctx_grp(pair32, ctx32_t, 0)
            vt_tile(2)
            st_exp(pair32, 2)
            den_grp(pair32, den32_t, 1, 16)
            ctx_grp(pair32, ctx32_t, 1)
            vt_tile(3)
            st_exp(pair32, 3)
            den_grp(pair32, den32_t, 2, 16)
            ctx_grp(pair32, ctx32_t, 2)
            st_exp(pair10, 0)
            den_grp(pair32, den32_t, 3, 16)
            ctx_grp(pair32, ctx32_t, 3)
            den_shared_finish(pair32, den32_t, den32_sb)
            nc.vector.reciprocal(rden[:, 6:8], dent[:, 6:8])
            ctx_finish(pair32, ctx32_t)
            out_block(4, nc.sync)
            out_block(5, nc.gpsimd)
            st_exp(pair10, 1)
            den10_t = p2.tile([P, 512], F32, tag="den", name="den10")
            # per-slot ctx banks: slot0 completes at g0 -> its outputs
            # unblock before slot1's solo group finishes
            ctx10a = p2.tile([P, 512], F32, tag="ctx", name="ctx10a")
            ctx10b = p2.tile([P, 512], F32, tag="vtop", bufs=2,
                             name="ctx10b")
            den_grp(pair10, den10_t, 0, 4)
            for kb in range(4):
                o = pt_off(1, kb)
                nc.tensor.matmul(ctx10a[:, 0:QW], lhsT=v_sb[:, kb, :],
                                 rhs=pt10_sb[:, o:o + QW],
                                 start=(kb == 0), stop=(kb == 3),
                                 skip_group_check=True)
                nc.tensor.matmul(ctx10b[:, 0:QW], lhsT=v_sb[:, kb, :],
                                 rhs=pt10_sb[:, o + QW:o + 512],
                                 start=(kb == 0), stop=False,
                                 skip_group_check=True)
            den_shared_finish(pair10, den10_t, den10_sb)
            nc.vector.tensor_copy(ctxt_sb[:, 0:QW], ctx10a[:, 0:QW])
            out_block(6, nc.scalar)
            out_block(7, nc.sync)
            out_block(0, nc.gpsimd)
            out_block(1, nc.scalar)
            # slot1 solo group: ctx + transposed den
            for kb in range(4, 8):
                o = pt_off(1, kb)
                nc.tensor.matmul(ctx10b[:, 0:QW], lhsT=v_sb[:, kb, :],
                                 rhs=pt10_sb[:, o:o + QW],
                                 start=False, stop=(kb == 7),
                                 skip_group_check=True)
            dentp10 = p2.tile([P, 512], F32, tag="ctx", name="dentp10")
            den_solo(pair10, dentp10)
            den_solo_finish(pair10, dentp10)
            nc.scalar.copy(ctxt_sb[:, QW:2 * QW], ctx10b[:, 0:QW])
            out_block(2, nc.sync, tag="st")
            out_block(3, nc.scalar, tag="st")


_NC_CACHE = None


def build_nc() -> bass.Bass:
    global _NC_CACHE
    if _NC_CACHE is not None:
        return _NC_CACHE
    nc = bacc.Bacc("TRN2", target_bir_lowering=False, debug=False)
    xt_d = nc.dram_tensor("xt", [D_MODEL, SEQ], F16, kind="ExternalInput").ap()
    wqk_d = nc.dram_tensor("wqk", [P, 2048], F16, kind="ExternalInput").ap()
    wv_d = nc.dram_tensor("wv", [P, 1024], F16, kind="ExternalInput").ap()
    womask_d = nc.dram_tensor("womask", [P, 4096], F16,
                              kind="ExternalInput").ap()
    out_d = nc.dram_tensor("out", [NQ, D_MODEL], F16,
                           kind="ExternalOutput").ap()
    with tile.TileContext(nc) as tc:
        _attention_kernel(tc, xt_d, wqk_d, wv_d, womask_d, out_d)
    nc.compile()
    _NC_CACHE = nc
    return nc


def _chunk_major(w):
    """[1024, 128] -> [128, 8*128]: row p holds chunks c of w[128c+p, :]."""
    return np.ascontiguousarray(
        w.reshape(DC, P, D_HEAD).transpose(1, 0, 2).reshape(P, DC * D_HEAD))


def kernel(x, wq, wk, wv, wo, _trace=False, _trace_kwargs=None):
    x = np.asarray(x, dtype=np.float32)
    wq = np.asarray(wq, dtype=np.float32)
    wk = np.asarray(wk, dtype=np.float32)
    wv = np.asarray(wv, dtype=np.float32)
    wo = np.asarray(wo, dtype=np.float32)

    nc = build_nc()

    wqk = np.concatenate(
        [_chunk_major(wq.astype(np.float16)),
         _chunk_major(wk.astype(np.float16))], axis=1)
    wvp = _chunk_major(wv.astype(np.float16))
    womasks = {}
    for parity in (0, 1):
        m = make_mask01(parity)                       # [512 k, 256 q] 0/1
        ms = ((m.reshape(4, P, QW).transpose(1, 0, 2) - 1.0)
              * 30000.0).astype(np.float16)           # additive 0/-30000
        mp = np.zeros((P, 4, 512), dtype=np.float16)
        mp[:, :, 0:QW] = ms                           # lower slot masked
        wom = np.empty((P, 4096), dtype=np.float16)
        wom[:, OFF_WO:OFF_WO + 1024] = wo.astype(np.float16)
        wom[:, OFF_MP:OFF_MP + 2048] = mp.reshape(P, 2048)
        wom[:, OFF_MS:OFF_MS + 1024] = ms.reshape(P, 1024)
        womasks[parity] = wom

    in_maps = []
    for core in range(NCORES):
        b, parity = core // 2, core % 2
        order = block_order(parity)
        perm = np.concatenate([np.arange(P) + P * o for o in order])
        xt = np.ascontiguousarray(x[b][perm, :].T.astype(np.float16))
        in_maps.append({"xt": xt, "wqk": wqk, "wv": wvp,
                        "womask": womasks[parity]})

    res = run_bass_kernel_spmd(
        nc, in_maps, core_ids=list(range(NCORES)),
        trace=_trace, **(_trace_kwargs or {}))

    out = np.empty_like(x)
    for core in range(NCORES):
        b, parity = core // 2, core % 2
        order = block_order(parity)
        core_out = res.results[core]["out"]
        for j in range(NSLOT):
            for i in range(2):
                qb = order[4 * j + i]
                out[b, P * qb:P * (qb + 1), :] = \
                    core_out[QW * j + P * i:QW * j + P * (i + 1), :].astype(
                        np.float32)
    if _trace:
        return out, res
    return out


# revision 17
# speedup vs baseline: 1.0311x; 1.0311x over previous
"""Single-head causal attention on 8 TRN2 NeuronCores (Bass/Tile), v3.

Problem: x [4, 2048, 1024] fp32; wq/wk/wv [1024, 128]; wo [128, 1024].
out = softmax_causal((x@wq)(x@wk)^T / sqrt(128)) @ (x@wv) @ wo

Sharding: 8 cores = 4 batches x 2 query-interleavings (causal-load-balanced
"zebra": per group of 4 seq blocks, even core takes {4g, 4g+3}, odd core
{4g+1, 4g+2}). Each core's x arrives transposed and column-permuted; slot j
(256 queries) attends to permuted key prefix [0 : 512*(j+1)] with a static
multiplicative fp16 mask on the diagonal 512-key group.

Key structure (from NTFF profiling of v1/v2):
- Inputs stream on all 3 HWDGE/SWDGE queues (sync/scalar/gpsimd), one
  512KB chunk per transfer, interleaved so each chunk lands just before
  the d-chunk-outer projection loop consumes it (~190 GB/s per queue).
- ~3us of dummy N=64 matmuls during the DMA wait trip the HAM clock gate
  so real matmuls run at 2.4 GHz from the first projection.
- Attention streams 512 cols/matmul over slot pairs (3,2) then (1,0) --
  LDWEIGHTS (~104ns) hides under the 213ns rhs stream. Solo (diagonal)
  groups store two 256-wide blocks contiguously so ACT/DVE ops stay 2D.
- V[k,h] comes from two dma_start_transpose calls (xbar), zero PE cost.
- den: ones-lhsT matmuls for shared groups (evacuated + transposed via
  4 small DMAs + reciprocal EARLY, off the critical path); the final solo
  group of each pair instead uses transposed matmuls (lhsT=PT block,
  rhs=ones -> [128q, 1]) so no [1,N] transpose sits on the tail.
- Output fp16; out-projection PSUM reuses the VT accumulator banks.

PSUM budget: P1 = warm 1 + qt 2 + kt 4 = 7 banks.
P2 = vt/out 2 + st 2x2 + den 1 + ctx 1 = 8 banks.
"""

import numpy as np

import concourse.bass as bass
from concourse import bacc
import concourse.mybir as mybir
import concourse.tile as tile
from concourse.bass_utils import run_bass_kernel_spmd
from concourse.masks import make_identity

F32 = mybir.dt.float32
F16 = mybir.dt.float16
F8 = mybir.dt.float8e4

D_MODEL = 1024
D_HEAD = 128
SEQ = 2048
BATCH = 4
NCORES = 8
P = 128            # partitions / block size
DC = D_MODEL // P  # 8 d_model chunks
NB = SEQ // P      # 16 seq blocks
NSLOT = 4          # query slots per core
QW = 256           # queries per slot
NQ = NSLOT * QW    # 1024 queries per core
SCALE = 1.0 / float(np.sqrt(D_HEAD))
EXP_BIAS = -6.5    # exp(scale*s - 6.5) <= e^5 ~ 148 < 240 (fp8e4-safe; max scaled score ~11.3)

# womask column offsets (fp16, [128, 4096])
OFF_WO = 0
OFF_MP = 1024            # pair mask   [128, 4*512]
OFF_MS = 3072            # solo mask   [128, 4*256]


def block_order(parity: int) -> list[int]:
    order = []
    for g in range(4):
        if parity == 0:
            order += [4 * g, 4 * g + 3, 4 * g + 1, 4 * g + 2]
        else:
            order += [4 * g + 1, 4 * g + 2, 4 * g, 4 * g + 3]
    return order


def make_mask01(parity: int) -> np.ndarray:
    """Multiplicative 0/1 mask for the diagonal 512-key group, applied to
    PT (post-exp), transposed: [512 k, 256 q] fp16."""
    P4 = block_order(parity)[:4]
    m = np.zeros((512, 256), dtype=np.float16)
    kr = np.arange(P)[:, None]
    qc = np.arange(P)[None, :]
    tri = (kr <= qc).astype(np.float16)
    for kb2 in range(4):
        K = P4[kb2]
        for qb2 in range(2):
            Q = P4[qb2]
            blk = m[P * kb2:P * (kb2 + 1), P * qb2:P * (qb2 + 1)]
            if K < Q:
                blk[:] = 1.0
            elif K > Q:
                blk[:] = 0.0
            else:
                blk[:] = tri
    return m


def _attention_kernel(tc: tile.TileContext, xt_d, wqk_d, wv_d, womask_d,
                      out_d):
    nc = tc.nc

    with (
        tc.tile_pool(name="const", bufs=1) as const_pool,
        tc.tile_pool(name="big", bufs=1) as big_pool,
        tc.tile_pool(name="outp", bufs=3) as out_pool,
    ):
        # ---- staggered input DMAs on 3 queues; chunk k lands just before
        # the projection loop needs it ----
        wqk_sb = const_pool.tile([P, 2048], F16)
        wv_sb = const_pool.tile([P, 1024], F16)
        womask_sb = const_pool.tile([P, 4096], F16)
        xt_sb = big_pool.tile([P, DC, SEQ], F16)

        def xchunk(queue, c, h):
            queue.dma_start(
                out=xt_sb[:, c, 1024 * h:1024 * (h + 1)],
                in_=xt_d[P * c:P * (c + 1), 1024 * h:1024 * (h + 1)])

        # Queue assignment follows measured per-queue rates (~120-140 GB/s
        # each, concurrent; SWDGE delivers from ~16us). 0.25MB granularity
        # keeps every PE input-stall under the ~3.4us HAM re-throttle
        # window. Consumption order is (c,h) lexicographic.
        nc.sync.dma_start(out=wqk_sb, in_=wqk_d)
        for c, h, q in [(0, 0, nc.scalar), (0, 1, nc.sync),
                        (1, 0, nc.scalar), (1, 1, nc.sync),
                        (2, 0, nc.scalar), (2, 1, nc.sync),
                        (3, 0, nc.gpsimd), (3, 1, nc.scalar),
                        (4, 0, nc.sync), (4, 1, nc.gpsimd),
                        (5, 0, nc.scalar), (5, 1, nc.sync),
                        (6, 0, nc.gpsimd), (6, 1, nc.scalar),
                        (7, 0, nc.sync), (7, 1, nc.scalar)]:
            xchunk(q, c, h)
            if (c, h) == (2, 1):
                nc.sync.dma_start(out=wv_sb, in_=wv_d)
        nc.scalar.dma_start(out=womask_sb, in_=womask_d)
        # dummy exp pinned behind the wqk DMA (dep forces the scheduler to
        # keep it -- and the ~1.5us ACT table load walrus attaches to it --
        # after the scalar-queue input issues instead of hoisting both to
        # the queue head, which would delay c0)
        expbias = const_pool.tile([P, 1], F32)
        nc.vector.memset(expbias, EXP_BIAS)
        actwarm = const_pool.tile([P, 1], F32)
        nc.scalar.activation(out=actwarm, in_=wqk_sb[:, 0:1],
                             func=mybir.ActivationFunctionType.Exp)

        wq_c = wqk_sb[:, 0:1024].rearrange("p (c h) -> p c h", h=P)
        wk_c = wqk_sb[:, 1024:2048].rearrange("p (c h) -> p c h", h=P)
        wv_c = wv_sb.rearrange("p (c h) -> p c h", h=P)
        wo_sb = womask_sb[:, OFF_WO:OFF_WO + 1024]
        maskp3 = womask_sb[:, OFF_MP:OFF_MP + 2048].rearrange(
            "p (b w) -> p b w", w=512)   # additive 0/-30000, [128, 4, 512]
        masks3 = womask_sb[:, OFF_MS:OFF_MS + 1024].rearrange(
            "p (b w) -> p b w", w=QW)    # additive, [128, 4, 256]

        # ---- constants ----
        ones = const_pool.tile([P, 1], F16)
        nc.vector.memset(ones, 1.0)
        ident = const_pool.tile([P, P], F16)
        make_identity(nc, ident)
        warm_w = const_pool.tile([P, 64], F16)
        nc.vector.memset(warm_w, 0.0)

        qt_sb = big_pool.tile([P, NQ], F16)
        kt_sb = big_pool.tile([P, SEQ], F16)
        vt_sb = big_pool.tile([P, SEQ], F16)
        v_sb = big_pool.tile([P, NB, P], F16)   # V[k,h]; block kb at [:, kb, :]
        ctxt_sb = big_pool.tile([P, NQ], F16)
        # flat PT slabs; shared group g: block kb at [512kb : 512kb+512];
        # solo groups: two 256-wide blocks packed per 512 entry
        pt32_sb = big_pool.tile([P, 16 * 512], F16)
        pt10_sb = big_pool.tile([P, 8 * 512], F16)
        den32_sb = big_pool.tile([1, 512], F32)
        den10_sb = big_pool.tile([1, 512], F32)
        dent = big_pool.tile([P, 8], F32)     # den, col = out block qbi
        dentmm = big_pool.tile([P, 8], F32)   # solo-group den part
        rden = big_pool.tile([P, 8], F32)

        def pt_off(pair_ngs, kb):
            """Column offset of block kb in its pair's PT slab."""
            nsh = 4 * pair_ngs   # blocks in shared groups
            if kb < nsh:
                return 512 * kb
            return 512 * nsh + QW * (kb - nsh)

        # ---- phase 1: PE warm-up burst, then QT + KT (d-chunk outer) ----
        with tc.tile_pool(name="p1", bufs=1, space="PSUM") as p1:
            warm_ps = p1.tile([P, 64], F32, tag="warm")
            for _ in range(76):
                nc.tensor.matmul(warm_ps[0:64, :], lhsT=warm_w, rhs=warm_w,
                                 start=True, stop=True, skip_group_check=True)
            qt_ps = [p1.tile([P, 512], F32, name=f"qt_ps{i}", tag=f"qt{i}")
                     for i in range(2)]
            kt_ps = [p1.tile([P, 512], F32, name=f"kt_ps{i}", tag=f"kt{i}")
                     for i in range(4)]
            for c in range(DC):
                xc = xt_sb[:, c, :]
                xr = xc.rearrange("p (g q) -> p g q", q=QW)
                # per half-chunk: one QT + two KT matmuls (matches the
                # 0.25MB DMA granularity)
                for h in range(2):
                    nc.tensor.matmul(qt_ps[h], lhsT=wq_c[:, c, :],
                                     rhs=xr[:, 4 * h:4 * h + 3:2, :],
                                     start=(c == 0), stop=(c == DC - 1),
                                     skip_group_check=True)
                    for t in (2 * h, 2 * h + 1):
                        nc.tensor.matmul(kt_ps[t], lhsT=wk_c[:, c, :],
                                         rhs=xc[:, 512 * t:512 * (t + 1)],
                                         start=(c == 0), stop=(c == DC - 1),
                                         skip_group_check=True)
            # split evacuations across DVE and ACT: the next PSUM pool's
            # first tiles gate on ALL of these (pool boundary)
            nc.vector.tensor_copy(qt_sb[:, 0:512], qt_ps[0])
            nc.scalar.copy(qt_sb[:, 512:1024], qt_ps[1])
            for t in range(4):
                if t % 2 == 0:
                    nc.vector.tensor_copy(kt_sb[:, 512 * t:512 * (t + 1)],
                                          kt_ps[t])
                else:
                    nc.scalar.copy(kt_sb[:, 512 * t:512 * (t + 1)], kt_ps[t])

        # ---- phase 2: VT (tile-outer) + xbar V-transpose + paired attention
        with tc.tile_pool(name="p2", bufs=1, space="PSUM") as p2:

            def vt_tile(t):
                ps = p2.tile([P, 512], F32, tag="vtop", bufs=2,
                             name=f"vt_ps{t}")
                for c in range(DC):
                    nc.tensor.matmul(ps, lhsT=wv_c[:, c, :],
                                     rhs=xt_sb[:, c, 512 * t:512 * (t + 1)],
                                     start=(c == 0), stop=(c == DC - 1),
                                     skip_group_check=True)
                nc.vector.tensor_copy(vt_sb[:, 512 * t:512 * (t + 1)], ps)
                if t % 2 == 1:
                    # V[k,h] for blocks 8t' .. 8t'+7 in one xbar call
                    tt = t // 2
                    nc.sync.dma_start_transpose(
                        out=v_sb[:, 8 * tt:8 * tt + 8, :],
                        in_=vt_sb[:, 1024 * tt:1024 * (tt + 1)])

            # pair = (qlo, ngs, pt, qbase)
            pair32 = (512, 3, pt32_sb, 4)
            pair10 = (0, 1, pt10_sb, 0)

            def st_exp(pair, g):
                """Scores (+ additive causal mask on the diagonal group,
                via identity-weighted matmuls) + exp, fp8 out."""
                qlo, ngs, pt, _ = pair
                solo = g >= ngs
                diag_p = g == ngs - 1   # shared diagonal: lower slot masked
                diag_s = solo           # solo diagonal: upper slot masked
                rhs = qt_sb[:, qlo + QW:qlo + 512] if solo \
                    else qt_sb[:, qlo:qlo + 512]
                for half in range(2):
                    st = p2.tile([P, 1024], F32, tag="st", bufs=2,
                                 name=f"st{pair[0]}_{g}_{half}")
                    for k2 in range(2):
                        kb = 4 * g + 2 * half + k2
                        w = QW if solo else 512
                        # solo: both MMs share one bank -- only the first
                        # may carry start=True (bank-wide has_written clear)
                        nc.tensor.matmul(st[:, w * k2:w * (k2 + 1)],
                                         lhsT=kt_sb[:, P * kb:P * (kb + 1)],
                                         rhs=rhs,
                                         start=(k2 == 0) if solo else True,
                                         stop=not (diag_p or diag_s),
                                         skip_group_check=True)
                    if diag_p:
                        for k2 in range(2):
                            nc.tensor.matmul(
                                st[:, 512 * k2:512 * (k2 + 1)],
                                lhsT=ident,
                                rhs=maskp3[:, 2 * half + k2, :],
                                start=False, stop=(k2 == 1),
                                skip_group_check=True)
                    if diag_s:
                        nc.tensor.matmul(
                            st[:, 0:512], lhsT=ident,
                            rhs=masks3[:, 2 * half:2 * half + 2, :],
                            start=False, stop=True,
                            skip_group_check=True)
                    kb0 = 4 * g + 2 * half
                    ww = 512 if solo else 1024   # contiguous in both cases
                    nc.scalar.activation(
                        out=pt[:, pt_off(ngs, kb0):pt_off(ngs, kb0) + ww],
                        in_=st[:, 0:ww],
                        func=mybir.ActivationFunctionType.Exp,
                        bias=expbias, scale=SCALE)

            def den_grp(pair, den_t, g, nkb):
                """ones-lhsT den matmuls (solo groups hit only the upper
                slot's 256 columns)."""
                _, ngs, pt, _ = pair
                solo = g >= ngs
                w = QW if solo else 512
                co = QW if solo else 0
                for k2 in range(4):
                    kb = 4 * g + k2
                    o = pt_off(ngs, kb)
                    nc.tensor.matmul(den_t[0:1, co:co + w], lhsT=ones,
                                     rhs=pt[:, o:o + w],
                                     start=(kb == 0), stop=(kb == nkb - 1),
                                     skip_group_check=True)

            def ctx_grp(pair, ctx_t, g):
                _, ngs, pt, _ = pair
                nkb = 4 * (ngs + 1)
                solo = g >= ngs
                w = QW if solo else 512
                co = QW if solo else 0
                for k2 in range(4):
                    kb = 4 * g + k2
                    o = pt_off(ngs, kb)
                    nc.tensor.matmul(ctx_t[:, co:co + w],
                                     lhsT=v_sb[:, kb, :],
                                     rhs=pt[:, o:o + w],
                                     start=(kb == 0), stop=(kb == nkb - 1),
                                     skip_group_check=True)

            def den_shared_finish(pair, den_t, den_row_sb):
                """Evacuate shared-group den sums, transpose via 4 small
                DMAs, reciprocal for the lower slot (fully summed)."""
                qbase = pair[3]
                nc.vector.tensor_copy(den_row_sb, den_t[0:1, 0:512])
                for m in range(4):
                    nc.sync.dma_start(
                        out=dent[:, qbase + m:qbase + m + 1],
                        in_=den_row_sb[:, P * m:P * (m + 1)])
                nc.vector.reciprocal(rden[:, qbase:qbase + 2],
                                     dent[:, qbase:qbase + 2])

            def den_solo(pair, dent_t):
                """Transposed den matmuls for the pair's solo group:
                lhsT = PT [128k, 128q], rhs = ones -> [128q, 1]."""
                _, ngs, pt, _ = pair
                g = ngs   # solo group index: pair32 g=3, pair10 g=1
                for qc in range(2):
                    for k2 in range(4):
                        kb = 4 * g + k2
                        o = pt_off(ngs, kb) + P * qc
                        nc.tensor.matmul(dent_t[:, qc:qc + 1],
                                         lhsT=pt[:, o:o + P],
                                         rhs=ones,
                                         start=(k2 == 0), stop=(k2 == 3),
                                         skip_group_check=True)

            def den_solo_finish(pair, dent_t):
                hi = pair[3] + 2
                nc.vector.tensor_copy(dentmm[:, hi:hi + 2], dent_t[:, 0:2])
                nc.vector.tensor_add(dent[:, hi:hi + 2], dent[:, hi:hi + 2],
                                     dentmm[:, hi:hi + 2])
                nc.vector.reciprocal(rden[:, hi:hi + 2], dent[:, hi:hi + 2])

            def ctx_finish(pair, ctx_t):
                qlo = pair[0]
                nc.scalar.copy(ctxt_sb[:, qlo:qlo + 512], ctx_t)

            def out_block(qbi, dma_queue, tag="vtop"):
                """Output projection for 128-query block qbi, scaled by
                1/den, fp16, DMA out."""
                ot = out_pool.tile([P, D_MODEL], F16, tag="ot")
                rd = rden[:, qbi:qbi + 1]
                if tag == "st":   # retired scores banks: one 2-bank tile
                    ps = p2.tile([P, 1024], F32, tag="st", bufs=2,
                                 name=f"op{qbi}")
                    halves = [ps[:, 0:512], ps[:, 512:1024]]
                else:             # vt/out rotation: two 1-bank tiles
                    halves = [p2.tile([P, 512], F32, tag="vtop", bufs=2,
                                      name=f"op{qbi}_{h}") for h in range(2)]
                for h in range(2):
                    nc.tensor.matmul(halves[h],
                                     lhsT=ctxt_sb[:, P * qbi:P * (qbi + 1)],
                                     rhs=wo_sb[:, 512 * h:512 * (h + 1)],
                                     start=True, stop=True,
                                     skip_group_check=True)
                    if h == 0:
                        nc.vector.tensor_scalar_mul(
                            ot[:, 512 * h:512 * (h + 1)], halves[h], rd)
                    else:
                        nc.scalar.mul(ot[:, 512 * h:512 * (h + 1)], halves[h],
                                      rd)
                dma_queue.dma_start(out=out_d[P * qbi:P * (qbi + 1), :], in_=ot)

            # ---- interleaved emission (PE order) ----
            vt_tile(0)
            st_exp(pair32, 0)
            vt_tile(1)
            st_exp(pair32, 1)
            den32_t = p2.tile([P, 512], F32, tag="den", name="den32")
            ctx32_t = p2.tile([P, 512], F32, tag="ctx", name="ctx32")
            den_grp(pair32, den32_t, 0, 16)
            ctx_grp(pair32, ctx32_t, 0)
            vt_tile(2)
            st_exp(pair32, 2)
            den_grp(pair32, den32_t, 1, 16)
            ctx_grp(pair32, ctx32_t, 1)
            vt_tile(3)
            st_exp(pair32, 3)
            den_grp(pair32, den32_t, 2, 16)
            ctx_grp(pair32, ctx32_t, 2)
            st_exp(pair10, 0)
            den_grp(pair32, den32_t, 3, 16)
            ctx_grp(pair32, ctx32_t, 3)
            den_shared_finish(pair32, den32_t, den32_sb)
            nc.vector.reciprocal(rden[:, 6:8], dent[:, 6:8])
            ctx_finish(pair32, ctx32_t)
            out_block(4, nc.sync)
            out_block(5, nc.gpsimd)
            st_exp(pair10, 1)
            den10_t = p2.tile([P, 512], F32, tag="den", name="den10")
            # per-slot ctx banks: slot0 completes at g0 -> its outputs
            # unblock before slot1's solo group finishes
            ctx10a = p2.tile([P, 512], F32, tag="ctx", name="ctx10a")
            ctx10b = p2.tile([P, 1024], F32, tag="st", bufs=2, name="ctx10b")
            den_grp(pair10, den10_t, 0, 4)
            for kb in range(4):
                o = pt_off(1, kb)
                nc.tensor.matmul(ctx10a[:, 0:QW], lhsT=v_sb[:, kb, :],
                                 rhs=pt10_sb[:, o:o + QW],
                                 start=(kb == 0), stop=(kb == 3),
                                 skip_group_check=True)
                nc.tensor.matmul(ctx10b[:, 0:QW], lhsT=v_sb[:, kb, :],
                                 rhs=pt10_sb[:, o + QW:o + 512],
                                 start=(kb == 0), stop=False,
                                 skip_group_check=True)
            den_shared_finish(pair10, den10_t, den10_sb)
            nc.vector.tensor_copy(ctxt_sb[:, 0:QW], ctx10a[:, 0:QW])
            out_block(6, nc.scalar)
            out_block(7, nc.sync)
            out_block(0, nc.gpsimd)
            out_block(1, nc.scalar)
            # slot1 solo group: ctx + transposed den
            for kb in range(4, 8):
                o = pt_off(1, kb)
                nc.tensor.matmul(ctx10b[:, 0:QW], lhsT=v_sb[:, kb, :],
                                 rhs=pt10_sb[:, o:o + QW],
                                 start=False, stop=(kb == 7),
                                 skip_group_check=True)
            dentp10 = p2.tile([P, 512], F32, tag="ctx", name="dentp10")
            den_solo(pair10, dentp10)
            den_solo_finish(pair10, dentp10)
            nc.scalar.copy(ctxt_sb[:, QW:2 * QW], ctx10b[:, 0:QW])
            out_block(2, nc.sync, tag="st")
            out_block(3, nc.scalar, tag="st")


_NC_CACHE = None


def build_nc() -> bass.Bass:
    global _NC_CACHE
    if _NC_CACHE is not None:
        return _NC_CACHE
    nc = bacc.Bacc("TRN2", target_bir_lowering=False, debug=False)
    xt_d = nc.dram_tensor("xt", [D_MODEL, SEQ], F16, kind="ExternalInput").ap()
    wqk_d = nc.dram_tensor("wqk", [P, 2048], F16, kind="ExternalInput").ap()
    wv_d = nc.dram_tensor("wv", [P, 1024], F16, kind="ExternalInput").ap()
    womask_d = nc.dram_tensor("womask", [P, 4096], F16,
                              kind="ExternalInput").ap()
    out_d = nc.dram_tensor("out", [NQ, D_MODEL], F16,
                           kind="ExternalOutput").ap()
    with tile.TileContext(nc) as tc:
        _attention_kernel(tc, xt_d, wqk_d, wv_d, womask_d, out_d)
    nc.compile()
    _NC_CACHE = nc
    return nc


def _chunk_major(w):
    """[1024, 128] -> [128, 8*128]: row p holds chunks c of w[128c+p, :]."""
    return np.ascontiguousarray(
        w.reshape(DC, P, D_HEAD).transpose(1, 0, 2).reshape(P, DC * D_HEAD))


def kernel(x, wq, wk, wv, wo, _trace=False, _trace_kwargs=None):
    x = np.asarray(x, dtype=np.float32)
    wq = np.asarray(wq, dtype=np.float32)
    wk = np.asarray(wk, dtype=np.float32)
    wv = np.asarray(wv, dtype=np.float32)
    wo = np.asarray(wo, dtype=np.float32)

    nc = build_nc()

    wqk = np.concatenate(
        [_chunk_major(wq.astype(np.float16)),
         _chunk_major(wk.astype(np.float16))], axis=1)
    wvp = _chunk_major(wv.astype(np.float16))
    womasks = {}
    for parity in (0, 1):
        m = make_mask01(parity)                       # [512 k, 256 q] 0/1
        ms = ((m.reshape(4, P, QW).transpose(1, 0, 2) - 1.0)
              * 30000.0).astype(np.float16)           # additive 0/-30000
        mp = np.zeros((P, 4, 512), dtype=np.float16)
        mp[:, :, 0:QW] = ms                           # lower slot masked
        wom = np.empty((P, 4096), dtype=np.float16)
        wom[:, OFF_WO:OFF_WO + 1024] = wo.astype(np.float16)
        wom[:, OFF_MP:OFF_MP + 2048] = mp.reshape(P, 2048)
        wom[:, OFF_MS:OFF_MS + 1024] = ms.reshape(P, 1024)
        womasks[parity] = wom

    in_maps = []
    for core in range(NCORES):
        b, parity = core // 2, core % 2
        order = block_order(parity)
        perm = np.concatenate([np.arange(P) + P * o for o in order])
        xt = np.ascontiguousarray(x[b][perm, :].T.astype(np.float16))
        in_maps.append({"xt": xt, "wqk": wqk, "wv": wvp,
                        "womask": womasks[parity]})

    res = run_bass_kernel_spmd(
        nc, in_maps, core_ids=list(range(NCORES)),
        trace=_trace, **(_trace_kwargs or {}))

    out = np.empty_like(x)
    for core in range(NCORES):
        b, parity = core // 2, core % 2
        order = block_order(parity)
        core_out = res.results[core]["out"]
        for j in range(NSLOT):
            for i in range(2):
                qb = order[4 * j + i]
                out[b, P * qb:P * (qb + 1), :] = \
                    core_out[QW * j + P * i:QW * j + P * (i + 1), :].astype(
                        np.float32)
    if _trace:
        return out, res
    return out
